# revision 1
# baseline (speedup 1.0000x reference)
"""Multi-head attention (b=2, t=2048, E=1024, h=16) on 8 Trainium2 cores.

Sharding: tensor-parallel over heads - 2 heads per core. Each core computes
Q/K/V for its heads from the (replicated, pre-transposed) x, runs attention,
applies its slice of W_out, and emits a full-shape partial output. The host
sums the 8 partials.

Device-side layout tricks:
- Scores are computed TRANSPOSED (St[j, i], key index j on partitions), so
  softmax's sum-over-keys folds into the P@V matmul and the 2048x2048 P
  matrix is never transposed. Max-subtraction is skipped: |S/sqrt(d)| < 10
  for this problem (verified), so exp() is safe in fp32.
- K^T for both heads lives stacked in one [128, NI] tensor; score matmuls
  contract only their head's 64 partitions (no zero-padding needed).
- V^T tiles are 256 cols: [V_A(64)|ones(64)|V_B(64)|ones(64)]; each head's
  PV stationary is one contiguous 128-col slice whose ones half replicates
  the softmax denominator into psum partitions 64:127, making normalization
  a partition-aligned reciprocal+multiply on DVE (no broadcast matmuls).
- Weights arrive host-prelaid in SBUF layout (2KB DMA descriptors); x
  streams as [128,1024] chunk-pair tiles (2KB descriptors, half the ring
  instructions).

Phase 2 is ACT(exp)-bound; the scalar engine runs nothing but the exp train
there (all DMA issues live on the sync ring in phase 2).
"""

import numpy as np
import ml_dtypes

import concourse.bass as bass
import concourse.mybir as mybir
import concourse.tile as tile
from concourse import bacc
from concourse.bass_utils import run_bass_kernel_spmd

F32 = mybir.dt.float32
BF16 = mybir.dt.bfloat16
AF = mybir.ActivationFunctionType

B = 2          # batch
T = 2048       # tokens per batch
E = 1024       # embed
H = 16         # heads
D = 64         # head dim
NC = 8         # cores
HPC = H // NC  # heads per core = 2
NI = B * T     # 4096 flattened tokens
DK = float(D) ** 0.5

EC = E // 128        # 8 contraction chunks for projections
IC_ALL = NI // 512   # 8 moving chunks over all tokens
JT = T // 128        # 16 key tiles per batch


def _build_nc():
    nc = bacc.Bacc("TRN2", target_bir_lowering=False, debug=False,
                   enable_asserts=False)

    xT = nc.dram_tensor("xT", [E, NI], BF16, kind="ExternalInput")
    wqT = nc.dram_tensor("wqT", [128, E], BF16, kind="ExternalInput")
    wkT = nc.dram_tensor("wkT", [128, E], BF16, kind="ExternalInput")
    wvT = nc.dram_tensor("wvT", [128, E], BF16, kind="ExternalInput")
    woT = nc.dram_tensor("woT", [128, E], BF16, kind="ExternalInput")
    idin = nc.dram_tensor("idin", [128, 128], BF16, kind="ExternalInput")
    out = nc.dram_tensor("out", [NI, E], BF16, kind="ExternalOutput")

    with tile.TileContext(nc) as tc:
        with (
            tc.tile_pool(name="persist", bufs=1) as persist,
            tc.tile_pool(name="xt", bufs=16) as xt_pool,
            tc.tile_pool(name="vt", bufs=2) as vt_pool,
            tc.tile_pool(name="pt", bufs=3) as pt_pool,
            tc.tile_pool(name="norm", bufs=2) as norm_pool,
            tc.tile_pool(name="outc", bufs=4) as outc_pool,
        ):
            # ---- persistent SBUF tensors ----
            wq_sb = persist.tile([128, E], BF16, name="wq_sb")
            wk_sb = persist.tile([128, E], BF16, name="wk_sb")
            wv_sb = persist.tile([128, E], BF16, name="wv_sb")
            wo_sb = persist.tile([128, E], BF16, name="wo_sb")
            ident = persist.tile([128, 128], BF16, name="ident")
            qt_sb = persist.tile([128, NI], BF16, name="qt_sb")
            # K^T both heads stacked: partitions 0:64 head A, 64:128 head B
            ktp = persist.tile([128, NI], BF16, name="ktp")
            # V^T per 128-token tile, 256 cols: [V_A|ones|V_B|ones]
            va_sb = persist.tile([128, (JT * B) * 256], BF16, name="va_sb")
            # attention output (normalized, both heads) per batch
            ot_a = persist.tile([128, T], BF16, name="ot_a_v8")
            ot_b = persist.tile([128, T], BF16, name="ot_b")
            ots = [ot_a, ot_b]

            # ---- startup DMAs: first x tiles, then weights ----
            xt_tiles = {}

            def fetch_x(ip, e):
                t = xt_pool.tile([128, 1024], BF16, tag="xt")
                ring = nc.sync if e % 2 == 0 else nc.scalar
                ring.dma_start(
                    t[:], xT[e * 128:(e + 1) * 128,
                             ip * 1024:(ip + 1) * 1024])
                xt_tiles[(ip, e)] = t

            fetch_x(0, 0)
            fetch_x(0, 1)
            # split weight loads: e=0 columns first so the first matmuls
            # only wait on a 32KB transfer, the rest streams behind
            nc.sync.dma_start(wq_sb[:, 0:128], wqT[:, 0:128])
            nc.scalar.dma_start(wk_sb[:, 0:128], wkT[:, 0:128])
            nc.scalar.dma_start(wv_sb[:, 0:128], wvT[:, 0:128])
            nc.sync.dma_start(wq_sb[:, 128:E], wqT[:, 128:E])
            nc.scalar.dma_start(wk_sb[:, 128:E], wkT[:, 128:E])
            nc.scalar.dma_start(wv_sb[:, 128:E], wvT[:, 128:E])
            for e in range(2, EC):
                fetch_x(0, e)
            nc.sync.dma_start(ident[:], idin[:, :])
            nc.scalar.dma_start(wo_sb[:], woT[:, :])
            va_ones = va_sb[:].rearrange(
                "p (t g u) -> p t g u", g=2, u=128)[:, :, :, 64:128]
            nc.gpsimd.memset(va_ones, 1.0)

            # ---- phase 1: QKV projections (+ V transpose, pipelined) ----
            with tc.tile_pool(name="ps1", bufs=1, space="PSUM") as ps1:
                vt_done = []

                def emit_vtrans(i, vt_t):
                    with nc.allow_low_precision(reason="bf16 compute"):
                        for s in range(4):
                            tk = i * 4 + s  # global 128-token tile
                            ps_vt = ps1.tile([128, 128], BF16, tag="vtp",
                                             bufs=2)
                            nc.tensor.transpose(
                                ps_vt[:], vt_t[:, s * 128:(s + 1) * 128],
                                ident[:])
                            dst = va_sb[
                                :, tk * 256:(tk + 1) * 256].rearrange(
                                "p (g u) -> p g u", g=2)[:, :, 0:64]
                            srcv = ps_vt[:].rearrange("p (g c) -> p g c", g=2)
                            nc.vector.tensor_copy(dst, srcv)

                for i in range(IC_ALL):
                    ip, half = divmod(i, 2)
                    isl = slice(i * 512, (i + 1) * 512)
                    hsl = slice(half * 512, (half + 1) * 512)
                    ps_q = ps1.tile([128, 512], F32, tag="q", bufs=2)
                    ps_k = ps1.tile([128, 512], F32, tag="k", bufs=2)
                    ps_v = ps1.tile([128, 512], F32, tag="v", bufs=2)
                    for e in range(EC):
                        xt_t = xt_tiles[(ip, e)]
                        esl = slice(e * 128, (e + 1) * 128)
                        st, sp = e == 0, e == EC - 1
                        nc.tensor.matmul(ps_q[:], wq_sb[:, esl], xt_t[:, hsl],
                                         start=st, stop=sp,
                                         skip_group_check=True)
                        nc.tensor.matmul(ps_k[:], wk_sb[:, esl], xt_t[:, hsl],
                                         start=st, stop=sp,
                                         skip_group_check=True)
                        nc.tensor.matmul(ps_v[:], wv_sb[:, esl], xt_t[:, hsl],
                                         start=st, stop=sp,
                                         skip_group_check=True)
                        # prefetch next pair during the odd chunk
                        if half == 1 and ip + 1 < IC_ALL // 2:
                            fetch_x(ip + 1, e)
                        if e == 2 and vt_done:
                            emit_vtrans(i - 1, vt_done.pop())
                    with nc.allow_low_precision(reason="bf16 compute"):
                        nc.vector.tensor_copy(qt_sb[:, isl], ps_q[:])
                        nc.vector.tensor_copy(ktp[:, isl], ps_k[:])
                        vt_t = vt_pool.tile([128, 512], BF16, tag="vt")
                        nc.vector.tensor_copy(vt_t[:], ps_v[:])
                        vt_done.append(vt_t)
                if vt_done:
                    emit_vtrans(IC_ALL - 1, vt_done.pop())

            # ---- phase 2: attention + out-projection, per 512-col chunk.
            # Epilogue (normalize) and out-projection of chunk n are emitted
            # inside chunk n+1's j-loop so the PE never stalls on the DVE
            # normalization chain.
            # PSUM banks: s (3 bufs x [128,1024]f32, shared with outproj
            # targets = 6) + oA + oB
            with tc.tile_pool(name="ps2", bufs=1, space="PSUM") as ps2:
                chunks = [(bb, ic) for bb in range(B) for ic in range(T // 512)]
                pending = None

                def emit_norm(bb, ic, ps_oA, ps_oB):
                    # copy-first: four partition-shifted DVE copies are the
                    # ONLY readers of the oA/oB psum banks, so the next
                    # chunk's PV reuses them right away. Head A lands on
                    # partitions 0:64, head B on 64:128, so every SB+SB op
                    # downstream is partition-aligned (HW requirement).
                    cp = norm_pool.tile([128, 512], F32, tag="cp")
                    rs = norm_pool.tile([128, 512], F32, tag="rs")
                    nc.vector.tensor_copy(cp[0:64, :], ps_oA[0:64, :])
                    nc.vector.tensor_copy(rs[0:64, :], ps_oA[64:128, :])
                    nc.vector.tensor_copy(cp[64:128, :], ps_oB[0:64, :])
                    nc.vector.tensor_copy(rs[64:128, :], ps_oB[64:128, :])
                    rc = norm_pool.tile([128, 512], F32, tag="rc")
                    nc.vector.reciprocal_approx_fast(rc[:], rs[:])
                    ot2h = ots[bb]
                    icsl = slice(ic * 512, (ic + 1) * 512)
                    with nc.allow_low_precision(reason="bf16 attn out"):
                        nc.vector.tensor_mul(ot2h[:, icsl], cp[:], rc[:])

                def emit_outproj(p, k, drain=False):
                    # output tile shares the "s" psum tag ([128,1024], two
                    # matmul halves) -> one cast + one contiguous-row DMA
                    bb_p, ic_p = p
                    ot2h = ots[bb_p]
                    t0 = ic_p * 512 + k * 128
                    g0 = bb_p * T + t0
                    ps_out = ps2.tile([128, 1024], F32, tag="s", bufs=3)
                    for ec in range(2):
                        esl = slice(ec * 512, (ec + 1) * 512)
                        nc.tensor.matmul(
                            ps_out[:, esl], ot2h[:, t0:t0 + 128],
                            wo_sb[:, esl],
                            start=True, stop=True, skip_group_check=True)
                    oc = outc_pool.tile([128, 1024], BF16, tag="oc")
                    with nc.allow_low_precision(reason="bf16 out"):
                        if drain and k % 2 == 1:
                            # scalar is idle after the last exp; split the
                            # tail casts across ACT + DVE
                            nc.scalar.copy(oc[:], ps_out[:])
                        else:
                            nc.vector.tensor_copy(oc[:], ps_out[:])
                    oring = nc.scalar if (drain and k % 2 == 0) else nc.sync
                    oring.dma_start(out[g0:g0 + 128, :], oc[:])

                def emit_s_half(bb, ic, jp, head):
                    # 2 score matmuls + 1 exp for one head of the jp pair.
                    # Emission order (A-scores, A-PV, B-scores, B-PV per
                    # iteration) keeps the in-order PE stream from parking
                    # ready PV matmuls behind S matmuls that wait on a
                    # later exp.
                    gisl = slice(bb * T + ic * 512, bb * T + (ic + 1) * 512)
                    psl = slice(0, 64) if head == 0 else slice(64, 128)
                    ps_s = ps2.tile([128, 1024], F32, tag="s", bufs=3)
                    for h in range(2):
                        j = 2 * jp + h
                        jsl = slice((bb * JT + j) * 128,
                                    (bb * JT + j + 1) * 128)
                        hs = slice(h * 512, (h + 1) * 512)
                        nc.tensor.matmul(
                            ps_s[:, hs], ktp[psl, jsl], qt_sb[psl, gisl],
                            start=True, stop=True, skip_group_check=True)
                    p = pt_pool.tile([128, 1024], BF16,
                                     tag="pA" if head == 0 else "pB")
                    with nc.allow_low_precision(reason="bf16 probs"):
                        nc.scalar.activation(p[:], ps_s[:], AF.Exp,
                                             scale=1.0 / DK)
                    return p

                def emit_pv_half(bb, jp, p, ps_o, head):
                    for h in range(2):
                        j = 2 * jp + h
                        vb = (bb * JT + j) * 256 + head * 128
                        hs = slice(h * 512, (h + 1) * 512)
                        nc.tensor.matmul(
                            ps_o[:], va_sb[:, vb:vb + 128], p[:, hs],
                            start=(j == 0), stop=(j == JT - 1),
                            skip_group_check=True)

                # flat pipeline over all (chunk, jp) steps: the S/exp
                # lookahead crosses chunk boundaries so the ACT train never
                # starves at a chunk edge. Norm runs in its own chunk's
                # tail; outproj of chunk n runs at jp0-3 of chunk n+1.
                steps = [(bb, ic, jp) for bb, ic in chunks
                         for jp in range(JT // 2)]
                pA = emit_s_half(steps[0][0], steps[0][1], 0, 0)
                pB = emit_s_half(steps[0][0], steps[0][1], 0, 1)
                ps_oA = ps_oB = None
                for t, (bb, ic, jp) in enumerate(steps):
                    if jp == 0:
                        ps_oA = ps2.tile([128, 512], F32, tag="oA", bufs=1)
                        ps_oB = ps2.tile([128, 512], F32, tag="oB", bufs=1)
                    nstep = steps[t + 1] if t + 1 < len(steps) else None
                    if nstep is not None:
                        nA = emit_s_half(nstep[0], nstep[1], nstep[2], 0)
                    else:
                        nA = None
                    emit_pv_half(bb, jp, pA, ps_oA, 0)
                    if nstep is not None:
                        nB = emit_s_half(nstep[0], nstep[1], nstep[2], 1)
                    else:
                        nB = None
                    emit_pv_half(bb, jp, pB, ps_oB, 1)
                    pA, pB = nA, nB
                    if pending is not None and 2 <= jp <= 5:
                        # jp2 at the earliest: the pending chunk's norm
                        # chain (DVE) needs ~2 jp to finish; earlier OP
                        # matmuls would stall the in-order PE queue on it
                        emit_outproj(pending, jp - 2)
                        if jp == 5:
                            pending = None
                    if jp == JT // 2 - 1:
                        emit_norm(bb, ic, ps_oA, ps_oB)
                        pending = (bb, ic)
                # drain last chunk's out-projection
                for k in range(4):
                    emit_outproj(pending, k, drain=True)
    nc.compile()
    return nc


_CACHE = {}


def _get_nc():
    if "nc" not in _CACHE:
        _CACHE["nc"] = _build_nc()
    return _CACHE["nc"]


def _sb_layout(w):
    # [E, 128] -> SBUF layout [128, E]: sb[p, e*128+d] = w[e*128+p, d]
    return np.ascontiguousarray(
        w.reshape(EC, 128, 128).transpose(1, 0, 2).reshape(128, E))


def _prep_in_maps(x, W_qkv, W_out):
    bf16 = ml_dtypes.bfloat16
    xT = np.ascontiguousarray(x.reshape(NI, E).T).astype(bf16)
    dd = np.arange(D)
    ident = np.eye(128, dtype=bf16)
    in_maps = []
    for c in range(NC):
        heads = [c * HPC + k for k in range(HPC)]
        rq = np.concatenate([dd * 48 + 0 * 16 + hh for hh in heads])
        rk = np.concatenate([dd * 48 + 1 * 16 + hh for hh in heads])
        rv = np.concatenate([dd * 48 + 2 * 16 + hh for hh in heads])
        cols = slice(c * 128, (c + 1) * 128)
        in_maps.append({
            "xT": xT,
            "wqT": _sb_layout(W_qkv[rq].T).astype(bf16),
            "wkT": _sb_layout(W_qkv[rk].T).astype(bf16),
            "wvT": _sb_layout(W_qkv[rv].T).astype(bf16),
            "woT": np.ascontiguousarray(W_out[:, cols].T).astype(bf16),
            "idin": ident,
        })
    return in_maps


def run(x, W_qkv, W_out, trace=False, **spmd_kwargs):
    x = np.asarray(x, dtype=np.float32)
    W_qkv = np.asarray(W_qkv, dtype=np.float32)
    W_out = np.asarray(W_out, dtype=np.float32)
    nc = _get_nc()
    in_maps = _prep_in_maps(x, W_qkv, W_out)
    res = run_bass_kernel_spmd(nc, in_maps, core_ids=list(range(NC)),
                               trace=trace, **spmd_kwargs)
    acc = res.results[0]["out"].astype(np.float32)
    for c in range(1, NC):
        acc = acc + res.results[c]["out"]
    return acc.reshape(B, T, E), res


def kernel(x, W_qkv, W_out):
    out, _ = run(x, W_qkv, W_out)
    return out

